# revision 31
# baseline (speedup 1.0000x reference)
"""Chamfer loss kernel for Trainium2, 8 NeuronCores.

Strategy (data-parallel over B, 2 batches/core):
  - S = -d^2 = 2*k1.k2 - |k1|^2 - |k2|^2 computed on the tensor engine via an
    augmented K=13 matmul in float16 with an exact hi/lo (Dekker) split:
    f32-class precision at 1 cycle/row.
  - Two symmetric passes per batch: pass F tiles S as [m(128), n(512)]
    (forward rows), pass B tiles S^T as [n(128), m(512)] (backward rows).
  - Per 128-row tile: DVE tensor_scalar with op1=max accumulator fuses the
    PSUM->SBUF copy with per-chunk row maxima; a tiny reduce gives the row
    max (= -min d^2, exact).
  - sigma selection without argmin indices: a {0,1} mask marking row-max
    positions is built against the SAME pass's row max (exact compare;
    alternating ACT Sign anti-mask / GPSIMD tensor_scalar is_ge positive
    mask by row-tile parity to balance engines), DMA-transposed (batched
    32-block transpose), then PE-contracted with [sigma_hi, sigma_lo, 1]
    to produce [sum sigma at max, count] per opposite-side row.
  - Host: sigma_sel = (hi+lo)/cnt (positive rows) or complement via the
    total sigma sums (anti rows), then the tiny final loss math.
"""

import numpy as np
import ml_dtypes

import concourse.bacc as bacc
import concourse.mybir as mybir
from concourse.tile import TileContext
from concourse.bass_utils import run_bass_kernel_spmd
from concourse import bass_isa

B, D3, M, N = 16, 3, 4096, 4096
NCORES = 8
BL = B // NCORES  # batches per core = 2
EPS = 1e-12
KAUG = 13

MT = M // 128  # 32 row-tiles per pass
NT = N // 512  # 8 column chunks of 512
F32 = mybir.dt.float32
F16 = mybir.dt.float16
BF16 = mybir.dt.bfloat16

_CACHED_NC = None


def build_nc():
    nc = bacc.Bacc(None, target_bir_lowering=False)

    afwd = nc.dram_tensor("afwd", [BL, KAUG, M], F16, kind="ExternalInput")
    bfwd = nc.dram_tensor("bfwd", [BL, KAUG, N], F16, kind="ExternalInput")
    abwd = nc.dram_tensor("abwd", [BL, KAUG, N], F16, kind="ExternalInput")
    bbwd = nc.dram_tensor("bbwd", [BL, KAUG, M], F16, kind="ExternalInput")
    s1ones = nc.dram_tensor("s1ones", [BL, 128, 3 * MT], BF16, kind="ExternalInput")
    s2ones = nc.dram_tensor("s2ones", [BL, 128, 3 * MT], BF16, kind="ExternalInput")

    smaxf = nc.dram_tensor("smaxf", [BL, 128, 3 * MT], F32, kind="ExternalOutput")
    smaxb = nc.dram_tensor("smaxb", [BL, 128, 3 * MT], F32, kind="ExternalOutput")
    usef = nc.dram_tensor("self_", [BL, 128, 9 * MT], F32, kind="ExternalOutput")
    useb = nc.dram_tensor("selb_", [BL, 128, 9 * MT], F32, kind="ExternalOutput")

    with TileContext(nc) as tc:
        with (
            tc.tile_pool(name="aug", bufs=1) as aug_pool,
            tc.tile_pool(name="rows", bufs=3) as row_pool,
            tc.tile_pool(name="rmax", bufs=2) as rmax_pool,
            tc.tile_pool(name="smax", bufs=2) as smax_pool,
            tc.tile_pool(name="mask", bufs=3) as mask_pool,
            tc.tile_pool(name="sel", bufs=2) as sel_pool,
            tc.tile_pool(name="ps", bufs=2, space="PSUM") as psum_pool,
            tc.tile_pool(name="psel", bufs=1, space="PSUM") as psel_pool,
        ):
            for b in range(BL):
                a_f = aug_pool.tile([KAUG, M], F16, tag="a_f")
                b_f = aug_pool.tile([KAUG, N], F16, tag="b_f")
                a_b = aug_pool.tile([KAUG, N], F16, tag="a_b")
                b_b = aug_pool.tile([KAUG, M], F16, tag="b_b")
                s1o = aug_pool.tile([128, 3 * MT], BF16, tag="s1o")
                s2o = aug_pool.tile([128, 3 * MT], BF16, tag="s2o")
                nc.sync.dma_start(out=a_f[:], in_=afwd[b])
                nc.sync.dma_start(out=b_f[:], in_=bfwd[b])
                nc.sync.dma_start(out=a_b[:], in_=abwd[b])
                nc.sync.dma_start(out=b_b[:], in_=bbwd[b])
                nc.sync.dma_start(out=s1o[:], in_=s1ones[b])
                nc.sync.dma_start(out=s2o[:], in_=s2ones[b])

                # (lhsT, rhs, sigma-of-other-side, smax out, sel out)
                for lhsT, rhs, sother, smax_dram, sel_dram in (
                    (a_f, b_f, s2o, smaxf, usef),
                    (a_b, b_b, s1o, smaxb, useb),
                ):
                    smax_sb = smax_pool.tile([128, 3 * MT], F32, tag="smax_sb")
                    psel = psel_pool.tile([128, 9 * MT], F32, tag="psel")

                    C_OF = [0] * 12 + [1] * 12 + [2] * 8  # 128-block -> chunk

                    def emit_contracts(rt, maskt, psel=psel, sother=sother):
                        for cc in range(MT):
                            c = C_OF[cc]
                            nc.tensor.matmul(
                                psel[:, (rt * 3 + c) * 3:(rt * 3 + c + 1) * 3],
                                maskt[:, cc, :],
                                sother[:, cc * 3:(cc + 1) * 3],
                                start=(cc in (0, 12, 24)),
                                stop=(cc in (11, 23, 31)),
                            )

                    CHUNKS = [(0, 1536), (1536, 1536), (3072, 1024)]
                    pending = None
                    for rt in range(MT):  # row-tiles of this pass
                        rowbuf = row_pool.tile([128, 4096], F32, tag="rowbuf")
                        mask = mask_pool.tile([128, 4096], BF16, tag="mask")
                        for ci, (off, w) in enumerate(CHUNKS):
                            ps = psum_pool.tile([128, 1536], F32, tag="s")
                            for sub in range(w // 512):
                                nc.tensor.matmul(
                                    ps[:, sub * 512:(sub + 1) * 512],
                                    lhsT[:, rt * 128:(rt + 1) * 128],
                                    rhs[:, off + sub * 512:
                                        off + (sub + 1) * 512],
                                    start=True, stop=True,
                                )
                            sm = smax_sb[:, rt * 3 + ci:rt * 3 + ci + 1]
                            # fused copy + chunk row-max on DVE
                            nc.vector.tensor_scalar(
                                out=rowbuf[:, off:off + w],
                                in0=ps[:, 0:w], scalar1=0.0, scalar2=None,
                                op0=mybir.AluOpType.add,
                                op1=mybir.AluOpType.max,
                                accum_out=sm,
                            )
                            # chunk-local mask of chunk-max positions; the
                            # winning chunk is selected on the host from the
                            # shipped chunk maxima.  Parity alternation
                            # balances ACT vs GPSIMD.
                            if rt % 2 == 0:
                                # anti-mask: {0 at chunk max, 1 elsewhere}
                                nc.scalar.activation(
                                    out=mask[:, off:off + w],
                                    in_=rowbuf[:, off:off + w],
                                    func=mybir.ActivationFunctionType.Sign,
                                    bias=sm, scale=-1.0,
                                )
                            else:
                                # positive mask: {1 at chunk max, 0 else}
                                nc.gpsimd.tensor_scalar(
                                    out=mask[:, off:off + w],
                                    in0=rowbuf[:, off:off + w],
                                    scalar1=sm, scalar2=None,
                                    op0=mybir.AluOpType.is_ge,
                                )
                        maskt = mask_pool.tile([128, MT, 128], BF16, tag="maskt")
                        nc.sync.dma_start_transpose(out=maskt[:], in_=mask[:])
                        if pending is not None:
                            emit_contracts(*pending)
                        pending = (rt, maskt)
                    emit_contracts(*pending)

                    nc.sync.dma_start(out=smax_dram[b], in_=smax_sb[:])
                    sel_sb = sel_pool.tile([128, 9 * MT], F32, tag="sel_sb")
                    nc.scalar.copy(out=sel_sb[:], in_=psel[:])
                    nc.sync.dma_start(out=sel_dram[b], in_=sel_sb[:])
    nc.compile()
    return nc


def _get_nc():
    global _CACHED_NC
    if _CACHED_NC is None:
        _CACHED_NC = build_nc()
    return _CACHED_NC


def _split16(x):
    h = x.astype(np.float16)
    l = (x - h.astype(np.float32)).astype(np.float16)
    return h, l


def _splitbf(x):
    h = x.astype(ml_dtypes.bfloat16)
    l = (x - h.astype(np.float32)).astype(ml_dtypes.bfloat16)
    return h, l


def _prep_core_inputs(k1, k2, sig1, sig2):
    """k1,k2: [BL,3,4096] f32; sig1,sig2: [BL,4096] f32."""
    sq1 = np.sum(k1 * k1, axis=1)
    sq2 = np.sum(k2 * k2, axis=1)
    onesM = np.ones_like(sq1)[:, None, :]
    onesN = np.ones_like(sq2)[:, None, :]

    ah, al = _split16(2.0 * k1)
    bh, bl = _split16(k2)
    s1h, s1l = _split16(sq1)
    s2h, s2l = _split16(sq2)
    s1h, s1l = s1h[:, None, :], s1l[:, None, :]
    s2h, s2l = s2h[:, None, :], s2l[:, None, :]

    f16 = np.float16
    afwd = np.concatenate(
        [ah, ah, al, s1h.astype(f16), s1l.astype(f16),
         onesM.astype(f16), onesM.astype(f16)], axis=1).astype(f16)
    bfwd = np.concatenate(
        [bh, bl, bh, -onesN.astype(f16), -onesN.astype(f16),
         -s2h.astype(f16), -s2l.astype(f16)], axis=1).astype(f16)
    abh, abl = _split16(2.0 * k2)
    bbh, bbl = _split16(k1)
    abwd = np.concatenate(
        [abh, abl, abh, onesN.astype(f16), onesN.astype(f16),
         s2h.astype(f16), s2l.astype(f16)], axis=1).astype(f16)
    bbwd = np.concatenate(
        [bbh, bbh, bbl, -s1h.astype(f16), -s1l.astype(f16),
         -onesM.astype(f16), -onesM.astype(f16)], axis=1).astype(f16)

    def sig_layout(sig):
        hi, lo = _splitbf(sig)
        out = np.zeros((sig.shape[0], 128, 3 * MT), ml_dtypes.bfloat16)
        hi_r = hi.reshape(-1, MT, 128)
        lo_r = lo.reshape(-1, MT, 128)
        out[:, :, 0::3] = np.transpose(hi_r, (0, 2, 1))
        out[:, :, 1::3] = np.transpose(lo_r, (0, 2, 1))
        out[:, :, 2::3] = 1.0
        return out

    return {"afwd": afwd, "bfwd": bfwd, "abwd": abwd, "bbwd": bbwd,
            "s1ones": sig_layout(sig1), "s2ones": sig_layout(sig2)}


def run_device(keypoints1, keypoints2, sigma1, sigma2, trace=False):
    nc = _get_nc()
    in_maps = []
    for c in range(NCORES):
        sl = slice(c * BL, (c + 1) * BL)
        in_maps.append(_prep_core_inputs(
            np.asarray(keypoints1[sl]), np.asarray(keypoints2[sl]),
            np.asarray(sigma1[sl]), np.asarray(sigma2[sl]),
        ))
    return run_bass_kernel_spmd(nc, in_maps, list(range(NCORES)), trace=trace)


CHUNK_W = np.array([1536, 1536, 1024], np.float32)
CHUNK_OFF = [0, 1536, 3072]


def _decode_sel(sel, sig_other, mode):
    """sel: [128, 9*MT] per-chunk device sums; returns [4096, 3] per-chunk
    sigma candidates.  Even row-tiles carry anti {0,1} sums (complement via
    per-chunk totals), odd carry positive {1,0} sums."""
    s = sel.reshape(128, MT, 3, 3).transpose(1, 0, 2, 3).reshape(-1, 3, 3)
    hi = sig_other.astype(ml_dtypes.bfloat16).astype(np.float32)
    lo = (sig_other - hi).astype(ml_dtypes.bfloat16).astype(np.float32)
    tot_hi = np.array([hi[o:o + int(w)].astype(np.float64).sum()
                       for o, w in zip(CHUNK_OFF, CHUNK_W)], np.float32)
    tot_lo = np.array([lo[o:o + int(w)].astype(np.float64).sum()
                       for o, w in zip(CHUNK_OFF, CHUNK_W)], np.float32)
    rt_idx = np.repeat(np.arange(MT), 128)
    even = (rt_idx % 2 == 0)[:, None]
    cnt = np.where(even, CHUNK_W[None] - s[:, :, 2], s[:, :, 2])
    val = np.where(even, (tot_hi[None] - s[:, :, 0]) + (tot_lo[None] - s[:, :, 1]),
                   s[:, :, 0] + s[:, :, 1])
    return (val / np.maximum(cnt, 1.0)).astype(np.float32)


def _finish_host(results, sigma1, sigma2):
    sigma1 = np.asarray(sigma1)
    sigma2 = np.asarray(sigma2)
    fwd_terms = np.zeros((B, M), np.float32)
    bwd_terms = np.zeros((B, N), np.float32)
    rows = np.arange(M)
    for c in range(NCORES):
        r = results[c]
        for bl in range(BL):
            bg = c * BL + bl
            # [128, 4*MT] -> [4096, 4] chunk maxima; global = max, winner =
            # first chunk attaining it (matches argmin-first semantics)
            mf = r["smaxf"][bl].reshape(128, MT, 3).transpose(1, 0, 2)
            mf = mf.reshape(M, 3)
            mb = r["smaxb"][bl].reshape(128, MT, 3).transpose(1, 0, 2)
            mb = mb.reshape(N, 3)
            neg_d2_f = -mf.max(1)
            neg_d2_b = -mb.max(1)
            cf = mf.argmax(1)
            cb = mb.argmax(1)
            min_f = np.sqrt(np.maximum(neg_d2_f, EPS).astype(np.float32))
            min_b = np.sqrt(np.maximum(neg_d2_b, EPS).astype(np.float32))
            sig2sel = _decode_sel(r["self_"][bl], sigma2[bg], None)[rows, cf]
            sig1sel = _decode_sel(r["selb_"][bl], sigma1[bg], None)[rows, cb]
            sig_f = (sigma1[bg] + sig2sel) * np.float32(0.5)
            sig_b = (sigma2[bg] + sig1sel) * np.float32(0.5)
            fwd_terms[bg] = np.log(sig_f) + min_f / sig_f
            bwd_terms[bg] = np.log(sig_b) + min_b / sig_b
    loss = fwd_terms.mean(dtype=np.float32) + bwd_terms.mean(dtype=np.float32)
    return np.float32(loss)


def kernel(keypoints1, keypoints2, sigma1, sigma2):
    res = run_device(keypoints1, keypoints2, sigma1, sigma2)
    return _finish_host(res.results, sigma1, sigma2)


# revision 34
# speedup vs baseline: 1.0004x; 1.0004x over previous
"""Chamfer loss kernel for Trainium2, 8 NeuronCores.

Strategy (data-parallel over B, 2 batches/core):
  - S = -d^2 = 2*k1.k2 - |k1|^2 - |k2|^2 computed on the tensor engine via an
    augmented K=13 matmul in float16 with an exact hi/lo (Dekker) split:
    f32-class precision at 1 cycle/row.
  - Two symmetric passes per batch: pass F tiles S as [m(128), n(512)]
    (forward rows), pass B tiles S^T as [n(128), m(512)] (backward rows).
  - Per 128-row tile: DVE tensor_scalar with op1=max accumulator fuses the
    PSUM->SBUF copy with per-chunk row maxima; a tiny reduce gives the row
    max (= -min d^2, exact).
  - sigma selection without argmin indices: a {0,1} mask marking row-max
    positions is built against the SAME pass's row max (exact compare;
    alternating ACT Sign anti-mask / GPSIMD tensor_scalar is_ge positive
    mask by row-tile parity to balance engines), DMA-transposed (batched
    32-block transpose), then PE-contracted with [sigma_hi, sigma_lo, 1]
    to produce [sum sigma at max, count] per opposite-side row.
  - Host: sigma_sel = (hi+lo)/cnt (positive rows) or complement via the
    total sigma sums (anti rows), then the tiny final loss math.
"""

import numpy as np
import ml_dtypes

import concourse.bacc as bacc
import concourse.mybir as mybir
from concourse.tile import TileContext
from concourse.bass_utils import run_bass_kernel_spmd
from concourse import bass_isa

B, D3, M, N = 16, 3, 4096, 4096
NCORES = 8
BL = B // NCORES  # batches per core = 2
EPS = 1e-12
KAUG = 13

MT = M // 128  # 32 row-tiles per pass
NT = N // 512  # 8 column chunks of 512
F32 = mybir.dt.float32
F16 = mybir.dt.float16
BF16 = mybir.dt.bfloat16

_CACHED_NC = None


def build_nc():
    nc = bacc.Bacc(None, target_bir_lowering=False)

    afwd = nc.dram_tensor("afwd", [BL, KAUG, M], F16, kind="ExternalInput")
    bfwd = nc.dram_tensor("bfwd", [BL, KAUG, N], F16, kind="ExternalInput")
    abwd = nc.dram_tensor("abwd", [BL, KAUG, N], F16, kind="ExternalInput")
    bbwd = nc.dram_tensor("bbwd", [BL, KAUG, M], F16, kind="ExternalInput")
    s1ones = nc.dram_tensor("s1ones", [BL, 128, 3 * MT], BF16, kind="ExternalInput")
    s2ones = nc.dram_tensor("s2ones", [BL, 128, 3 * MT], BF16, kind="ExternalInput")

    smaxf = nc.dram_tensor("smaxf", [BL, 128, 3 * MT], F32, kind="ExternalOutput")
    smaxb = nc.dram_tensor("smaxb", [BL, 128, 3 * MT], F32, kind="ExternalOutput")
    usef = nc.dram_tensor("self_", [BL, 128, 9 * MT], F32, kind="ExternalOutput")
    useb = nc.dram_tensor("selb_", [BL, 128, 9 * MT], F32, kind="ExternalOutput")

    with TileContext(nc) as tc:
        with (
            tc.tile_pool(name="aug", bufs=1) as aug_pool,
            tc.tile_pool(name="rows", bufs=3) as row_pool,
            tc.tile_pool(name="rmax", bufs=2) as rmax_pool,
            tc.tile_pool(name="smax", bufs=2) as smax_pool,
            tc.tile_pool(name="mask", bufs=3) as mask_pool,
            tc.tile_pool(name="sel", bufs=2) as sel_pool,
            tc.tile_pool(name="ps", bufs=2, space="PSUM") as psum_pool,
            tc.tile_pool(name="psel", bufs=2, space="PSUM") as psel_pool,
        ):
            for b in range(BL):
                a_f = aug_pool.tile([KAUG, M], F16, tag="a_f")
                b_f = aug_pool.tile([KAUG, N], F16, tag="b_f")
                a_b = aug_pool.tile([KAUG, N], F16, tag="a_b")
                b_b = aug_pool.tile([KAUG, M], F16, tag="b_b")
                s1o = aug_pool.tile([128, 3 * MT], BF16, tag="s1o")
                s2o = aug_pool.tile([128, 3 * MT], BF16, tag="s2o")
                nc.sync.dma_start(out=a_f[:], in_=afwd[b])
                nc.sync.dma_start(out=b_f[:], in_=bfwd[b])
                nc.sync.dma_start(out=a_b[:], in_=abwd[b])
                nc.sync.dma_start(out=b_b[:], in_=bbwd[b])
                nc.sync.dma_start(out=s1o[:], in_=s1ones[b])
                nc.sync.dma_start(out=s2o[:], in_=s2ones[b])

                # (lhsT, rhs, sigma-of-other-side, smax out, sel out)
                for lhsT, rhs, sother, smax_dram, sel_dram in (
                    (a_f, b_f, s2o, smaxf, usef),
                    (a_b, b_b, s1o, smaxb, useb),
                ):
                    smax_sb = smax_pool.tile([128, 3 * MT], F32, tag="smax_sb")
                    sel_sb = sel_pool.tile([128, 9 * MT], F32, tag="sel_sb")

                    C_OF = [0] * 12 + [1] * 12 + [2] * 8  # 128-block -> chunk

                    def emit_contracts(rt, maskt, sel_sb=sel_sb, sother=sother):
                        psel_rt = psel_pool.tile([128, 9], F32, tag="psel_rt")
                        for cc in range(MT):
                            c = C_OF[cc]
                            nc.tensor.matmul(
                                psel_rt[:, c * 3:(c + 1) * 3],
                                maskt[:, cc, :],
                                sother[:, cc * 3:(cc + 1) * 3],
                                start=(cc in (0, 12, 24)),
                                stop=(cc in (11, 23, 31)),
                            )
                        nc.scalar.copy(
                            out=sel_sb[:, rt * 9:(rt + 1) * 9], in_=psel_rt[:]
                        )

                    CHUNKS = [(0, 1536), (1536, 1536), (3072, 1024)]
                    pending = None
                    for rt in range(MT):  # row-tiles of this pass
                        rowbuf = row_pool.tile([128, 4096], F32, tag="rowbuf")
                        mask = mask_pool.tile([128, 4096], BF16, tag="mask")
                        for ci, (off, w) in enumerate(CHUNKS):
                            ps = psum_pool.tile([128, 1536], F32, tag="s")
                            for sub in range(w // 512):
                                nc.tensor.matmul(
                                    ps[:, sub * 512:(sub + 1) * 512],
                                    lhsT[:, rt * 128:(rt + 1) * 128],
                                    rhs[:, off + sub * 512:
                                        off + (sub + 1) * 512],
                                    start=True, stop=True,
                                )
                            sm = smax_sb[:, rt * 3 + ci:rt * 3 + ci + 1]
                            # fused copy + chunk row-max on DVE
                            nc.vector.tensor_scalar(
                                out=rowbuf[:, off:off + w],
                                in0=ps[:, 0:w], scalar1=0.0, scalar2=None,
                                op0=mybir.AluOpType.add,
                                op1=mybir.AluOpType.max,
                                accum_out=sm,
                            )
                            # chunk-local mask of chunk-max positions; the
                            # winning chunk is selected on the host from the
                            # shipped chunk maxima.  Parity alternation
                            # balances ACT vs GPSIMD.
                            if rt % 2 == 0:
                                # anti-mask: {0 at chunk max, 1 elsewhere}
                                nc.scalar.activation(
                                    out=mask[:, off:off + w],
                                    in_=rowbuf[:, off:off + w],
                                    func=mybir.ActivationFunctionType.Sign,
                                    bias=sm, scale=-1.0,
                                )
                            else:
                                # positive mask: {1 at chunk max, 0 else}
                                nc.gpsimd.tensor_scalar(
                                    out=mask[:, off:off + w],
                                    in0=rowbuf[:, off:off + w],
                                    scalar1=sm, scalar2=None,
                                    op0=mybir.AluOpType.is_ge,
                                )
                        maskt = mask_pool.tile([128, MT, 128], BF16, tag="maskt")
                        nc.sync.dma_start_transpose(out=maskt[:], in_=mask[:])
                        if pending is not None:
                            emit_contracts(*pending)
                        pending = (rt, maskt)
                    emit_contracts(*pending)

                    nc.sync.dma_start(out=smax_dram[b], in_=smax_sb[:])
                    nc.sync.dma_start(out=sel_dram[b], in_=sel_sb[:])
    nc.compile()
    return nc


def _get_nc():
    global _CACHED_NC
    if _CACHED_NC is None:
        _CACHED_NC = build_nc()
    return _CACHED_NC


def _split16(x):
    h = x.astype(np.float16)
    l = (x - h.astype(np.float32)).astype(np.float16)
    return h, l


def _splitbf(x):
    h = x.astype(ml_dtypes.bfloat16)
    l = (x - h.astype(np.float32)).astype(ml_dtypes.bfloat16)
    return h, l


def _prep_core_inputs(k1, k2, sig1, sig2):
    """k1,k2: [BL,3,4096] f32; sig1,sig2: [BL,4096] f32."""
    sq1 = np.sum(k1 * k1, axis=1)
    sq2 = np.sum(k2 * k2, axis=1)
    onesM = np.ones_like(sq1)[:, None, :]
    onesN = np.ones_like(sq2)[:, None, :]

    ah, al = _split16(2.0 * k1)
    bh, bl = _split16(k2)
    s1h, s1l = _split16(sq1)
    s2h, s2l = _split16(sq2)
    s1h, s1l = s1h[:, None, :], s1l[:, None, :]
    s2h, s2l = s2h[:, None, :], s2l[:, None, :]

    f16 = np.float16
    afwd = np.concatenate(
        [ah, ah, al, s1h.astype(f16), s1l.astype(f16),
         onesM.astype(f16), onesM.astype(f16)], axis=1).astype(f16)
    bfwd = np.concatenate(
        [bh, bl, bh, -onesN.astype(f16), -onesN.astype(f16),
         -s2h.astype(f16), -s2l.astype(f16)], axis=1).astype(f16)
    abh, abl = _split16(2.0 * k2)
    bbh, bbl = _split16(k1)
    abwd = np.concatenate(
        [abh, abl, abh, onesN.astype(f16), onesN.astype(f16),
         s2h.astype(f16), s2l.astype(f16)], axis=1).astype(f16)
    bbwd = np.concatenate(
        [bbh, bbh, bbl, -s1h.astype(f16), -s1l.astype(f16),
         -onesM.astype(f16), -onesM.astype(f16)], axis=1).astype(f16)

    def sig_layout(sig):
        hi, lo = _splitbf(sig)
        out = np.zeros((sig.shape[0], 128, 3 * MT), ml_dtypes.bfloat16)
        hi_r = hi.reshape(-1, MT, 128)
        lo_r = lo.reshape(-1, MT, 128)
        out[:, :, 0::3] = np.transpose(hi_r, (0, 2, 1))
        out[:, :, 1::3] = np.transpose(lo_r, (0, 2, 1))
        out[:, :, 2::3] = 1.0
        return out

    return {"afwd": afwd, "bfwd": bfwd, "abwd": abwd, "bbwd": bbwd,
            "s1ones": sig_layout(sig1), "s2ones": sig_layout(sig2)}


def run_device(keypoints1, keypoints2, sigma1, sigma2, trace=False):
    nc = _get_nc()
    in_maps = []
    for c in range(NCORES):
        sl = slice(c * BL, (c + 1) * BL)
        in_maps.append(_prep_core_inputs(
            np.asarray(keypoints1[sl]), np.asarray(keypoints2[sl]),
            np.asarray(sigma1[sl]), np.asarray(sigma2[sl]),
        ))
    return run_bass_kernel_spmd(nc, in_maps, list(range(NCORES)), trace=trace)


CHUNK_W = np.array([1536, 1536, 1024], np.float32)
CHUNK_OFF = [0, 1536, 3072]


def _decode_sel(sel, sig_other, mode):
    """sel: [128, 9*MT] per-chunk device sums; returns [4096, 3] per-chunk
    sigma candidates.  Even row-tiles carry anti {0,1} sums (complement via
    per-chunk totals), odd carry positive {1,0} sums."""
    s = sel.reshape(128, MT, 3, 3).transpose(1, 0, 2, 3).reshape(-1, 3, 3)
    hi = sig_other.astype(ml_dtypes.bfloat16).astype(np.float32)
    lo = (sig_other - hi).astype(ml_dtypes.bfloat16).astype(np.float32)
    tot_hi = np.array([hi[o:o + int(w)].astype(np.float64).sum()
                       for o, w in zip(CHUNK_OFF, CHUNK_W)], np.float32)
    tot_lo = np.array([lo[o:o + int(w)].astype(np.float64).sum()
                       for o, w in zip(CHUNK_OFF, CHUNK_W)], np.float32)
    rt_idx = np.repeat(np.arange(MT), 128)
    even = (rt_idx % 2 == 0)[:, None]
    cnt = np.where(even, CHUNK_W[None] - s[:, :, 2], s[:, :, 2])
    val = np.where(even, (tot_hi[None] - s[:, :, 0]) + (tot_lo[None] - s[:, :, 1]),
                   s[:, :, 0] + s[:, :, 1])
    return (val / np.maximum(cnt, 1.0)).astype(np.float32)


def _finish_host(results, sigma1, sigma2):
    sigma1 = np.asarray(sigma1)
    sigma2 = np.asarray(sigma2)
    fwd_terms = np.zeros((B, M), np.float32)
    bwd_terms = np.zeros((B, N), np.float32)
    rows = np.arange(M)
    for c in range(NCORES):
        r = results[c]
        for bl in range(BL):
            bg = c * BL + bl
            # [128, 4*MT] -> [4096, 4] chunk maxima; global = max, winner =
            # first chunk attaining it (matches argmin-first semantics)
            mf = r["smaxf"][bl].reshape(128, MT, 3).transpose(1, 0, 2)
            mf = mf.reshape(M, 3)
            mb = r["smaxb"][bl].reshape(128, MT, 3).transpose(1, 0, 2)
            mb = mb.reshape(N, 3)
            neg_d2_f = -mf.max(1)
            neg_d2_b = -mb.max(1)
            cf = mf.argmax(1)
            cb = mb.argmax(1)
            min_f = np.sqrt(np.maximum(neg_d2_f, EPS).astype(np.float32))
            min_b = np.sqrt(np.maximum(neg_d2_b, EPS).astype(np.float32))
            sig2sel = _decode_sel(r["self_"][bl], sigma2[bg], None)[rows, cf]
            sig1sel = _decode_sel(r["selb_"][bl], sigma1[bg], None)[rows, cb]
            sig_f = (sigma1[bg] + sig2sel) * np.float32(0.5)
            sig_b = (sigma2[bg] + sig1sel) * np.float32(0.5)
            fwd_terms[bg] = np.log(sig_f) + min_f / sig_f
            bwd_terms[bg] = np.log(sig_b) + min_b / sig_b
    loss = fwd_terms.mean(dtype=np.float32) + bwd_terms.mean(dtype=np.float32)
    return np.float32(loss)


def kernel(keypoints1, keypoints2, sigma1, sigma2):
    res = run_device(keypoints1, keypoints2, sigma1, sigma2)
    return _finish_host(res.results, sigma1, sigma2)
